# revision 28
# baseline (speedup 1.0000x reference)
"""Trainium2 Bass kernel for DigitConvolutionalModel (8-core data parallel).

Computation: x(B,784) -> 3x3 valid conv on 28x28 -> flatten(676)
             -> FC(100)+ReLU -> FC(10), B = 65536.

Algebraic restructure (host side, exact): the conv is linear, so conv and
fc1 fold into one 784->100 matrix W1eff (accumulated in float64). The
device kernel is then just two matmul layers per 512-sample tile:
  h = relu(x @ W1eff + b1);  y = h @ fc2_w.T + b2.

Numerics: the matmul datapath runs in fp16 (inputs rounded once on the
host). Measured end-to-end scale-relative absmax error vs the fp32
reference is ~4.5e-4; fp16 streams the PE at 1 col/cycle (fp32 runs at
~1/4 rate) and halves the HBM traffic, which is the kernel's bottleneck.

DMA model (measured on this part):
  - a queue keeps ~2 descriptors in flight (issue n+2 waits on n);
  - per-descriptor byte rate is descriptor-row-size bound (the DGE emits
    one packet per row; bigger rows amortize the ~105ns/packet engine
    overhead), so full-width rows (6144B) stream ~2x the rate of
    column-half rows (3072B);
  - tiny-row descriptors (cb's 8B rows, w1r's 220B rows) are pathological
    and live on the otherwise-idle GPSIMD queue, as do the y flushes
    (their bias-wait must not head-of-line block an x ring).
Every x tile therefore moves as TWO partition-row halves (rows 0:64 on
the sync ring, 64:128 on the scalar ring): full 6144B rows, both rings
loaded identically, and the last tile drains both rings concurrently.

Per-core layout (B_shard=8192 = 16 tiles x 512): x is pre-transposed on
the host to feature-major tiles so the matmul contraction lands on SBUF
partitions with no on-device transposes. Features 0..767 form 6 chunks
of 128 partitions; the 16 remainder features for all 16 tiles are packed
into one [128, 2048] tile at 32-aligned partition groups (PE row-group
granularity) and applied with per-group w1r replicas. fc1/fc2 biases ride
as two extra fp16 columns of the cpk constant block. Outputs accumulate
in SBUF and leave in tapered gpsimd writebacks so only two small writes
remain after the final tile.
"""

import numpy as np

import concourse.bass as bass
import concourse.mybir as mybir
import concourse.tile as tile
from concourse.bass_utils import run_bass_kernel_spmd
from concourse.vector_clock import ScopedClock

N_CORES = 8
B_TOTAL = 65536
B_SHARD = B_TOTAL // N_CORES  # 8192
BT = 512  # batch tile (one PSUM bank of fp32)
N_TILES = B_SHARD // BT  # 16
FC = 6  # full 128-partition feature chunks (6*128 = 768)
F_REM = 784 - FC * 128  # 16 remainder features
H1 = 100
H2 = 10

_f32 = mybir.dt.float32
_f16 = mybir.dt.float16


class SplitDrainTileContext(tile.TileContext):
    """TileContext whose tail drain carries at most one sync wait.

    The pinned walrus rejects instructions with >2 sync waits
    ("Too many sync wait commands" in setupSyncWait); the stock tail
    drain accumulates one wait per active proc. Emit one drain per
    wait instead — consecutive drains on the sync engine are
    semantically equivalent to one drain carrying all the waits.
    """

    def _drain_and_barrier(self, tick_clock, wait_clock):
        nc = self.nc
        # Cheap tail: the stock version runs two full EVSEM butterflies
        # (~13us measured). Instead: gpsimd waits on the whole vector
        # clock (all tracked incs have landed), every engine drains its
        # own DGE queues, gpsimd clears the sem ranges, and one
        # sequencer-level sem-only barrier closes the kernel.
        drain_inst = nc.gpsimd.drain()
        wait_clock.add_sem_waits(
            drain_inst.ins, ScopedClock({None: tick_clock.global_clock})
        )
        raw = drain_inst.ins
        si = raw.sync_info
        if si is not None and si.on_wait and len(si.on_wait) > 1:
            waits = list(si.on_wait)
            si.on_wait = waits[:1]
            raw.sync_info = si
            for w in waits[1:]:
                extra = nc.gpsimd.drain()
                extra.ins.sync_info = mybir.SyncInfo(on_wait=[w], on_update=[])
        for eng in (nc.sync, nc.scalar, nc.vector, nc.tensor):
            eng.drain()

        # No tail barrier: gpsimd's global-clock waits above guarantee all
        # tracked sem incs (incl. DMA completions) have landed before the
        # clears, and NRT serializes re-executions on all-engine completion.
        assert self.sems is not None
        popped = nc._tile_sem_poison_stack.pop()
        assert popped is self._sem_poison
        nc.clear_and_free_semaphores(list(self.sems.allocated().values()))


def _split_sync_waits(nc: bass.Bass, limit: int = 1) -> None:
    """Walrus-compat post-pass: the pinned walrus rejects instructions
    carrying more than ~2 sync waits. Hoist excess waits onto NoOp
    instructions inserted just before the offending instruction on the
    same engine — semantically identical (waits run in stream order)."""
    n = 0
    for fn in nc.m.functions:
        for bb in fn.blocks:
            out = []
            changed = False
            for inst in bb.instructions:
                si = inst.sync_info
                if si is not None and si.on_wait and len(si.on_wait) > limit:
                    waits = list(si.on_wait)
                    for i in range(0, len(waits) - limit, limit):
                        nop = mybir.InstNoOp(
                            name=f"swsplit-{n}",
                            ins=[],
                            outs=[],
                            sync_info=mybir.SyncInfo(
                                on_wait=waits[i : i + limit], on_update=[]
                            ),
                        )
                        nop.engine = inst.engine
                        out.append(nop)
                        n += 1
                    si.on_wait = waits[len(waits) - limit :]
                    inst.sync_info = si
                    changed = True
                out.append(inst)
            if changed:
                bb.instructions = out


WARM_MM = 48  # bridges Tensor-ready (~7.4us) to w1m+x0c0-2 landed (~10.4us)

# mA stream: [w1m | x0 | xr], f16 columns -- xr rides the main sync
# stream at full rate instead of a contending third queue
XR_W = 4 * BT
MA_W = FC * H1 + FC * BT + XR_W  # 600 + 3072 + 2048
# mB stream: [x1 | cpk], f16 columns; cpk = [w1r | w2 | b1 | b2]
CPK_W = H1 + H2 + 2
MB_W = FC * BT + CPK_W


def _build_nc() -> bass.Bass:
    nc = bass.Bass(monotonic_sem_count=0)
    # tiles 2..15 feature-major (see _make_in_maps)
    xm = nc.dram_tensor("xm", [N_TILES - 2, 128, FC, BT], _f16, kind="ExternalInput")
    mA = nc.dram_tensor("mA", [128, MA_W], _f16, kind="ExternalInput")
    mB = nc.dram_tensor("mB", [128, MB_W], _f16, kind="ExternalInput")
    # packed fp32 consts: col 0 b1 (rows 0..99), col 1 b2 (rows 0..9) --
    # tensor_scalar requires fp32 scalar operands
    cb = nc.dram_tensor("cb", [128, 2], _f32, kind="ExternalInput")
    y = nc.dram_tensor("y", [H2, N_TILES * BT], _f32, kind="ExternalOutput")

    with SplitDrainTileContext(nc) as tc:
        with (
            tc.tile_pool(name="consts", bufs=1) as cpool,
            tc.tile_pool(name="xp", bufs=12) as xpool,
            tc.tile_pool(name="hp", bufs=4) as hpool,
            tc.tile_pool(name="psh", bufs=5, space="PSUM") as psh,
            tc.tile_pool(name="pso", bufs=3, space="PSUM") as pso,
        ):
            # head: w1m first (small, clears fast), then x0/x1 as row
            # halves split across both rings. cpk (tiny rows) and xr ride
            # the gpsimd queue so they never displace an x descriptor.
            mA_sb = cpool.tile([128, MA_W], _f16, tag="mA")
            mB_sb = cpool.tile([128, MB_W], _f16, tag="mB")
            W0 = FC * H1
            hw = FC * BT // 2
            # sync's first descriptor carries w1m AND x0's first chunks in
            # one region (4272B rows): w1m alone has 1200B rows and crawls.
            # Rings drain descriptors serially, so order = priority: sync
            # then streams xr (inline mA columns) behind x0h0; scalar
            # carries x0h1 then x1.
            X0E = W0 + FC * BT
            # x0 and x1 each split ACROSS rings so A0 is dense from ~10.5
            # (a sparse A0 never sustains the HAM clock-ramp window and
            # pairs 0-1 then run at 1.2 GHz). xr streams behind x1h0 on
            # sync -- in place before R0, without delaying x0/x1.
            nc.sync.dma_start(out=mA_sb[:, : W0 + hw], in_=mA[:, : W0 + hw])
            nc.scalar.dma_start(out=mA_sb[:, W0 + hw : X0E], in_=mA[:, W0 + hw : X0E])
            nc.sync.dma_start(out=mB_sb[:, :hw], in_=mB[:, :hw])
            nc.scalar.dma_start(out=mB_sb[:, hw : FC * BT], in_=mB[:, hw : FC * BT])
            nc.sync.dma_start(out=mA_sb[:, X0E:], in_=mA[:, X0E:])
            nc.scalar.dma_start(out=mB_sb[:, FC * BT :], in_=mB[:, FC * BT :])
            cb_sb = cpool.tile([128, 2], _f32, tag="cb")
            nc.scalar.dma_start(out=cb_sb[:], in_=cb[:])
            xr_sb = mA_sb[:, X0E:]

            w1m_sb = mA_sb[:, :W0]
            cpk = mB_sb[:, FC * BT :]
            w1r_sb = cpk[:, :H1]
            w2_sb = cpk[:H1, H1 : H1 + H2]
            b1_sb = cb_sb[:H1, 0:1]
            b2_sb = cb_sb[:H2, 1:2]
            x_sbs = {0: mA_sb[:, W0:X0E], 1: mB_sb[:, : FC * BT]}

            # outputs accumulate here; written back in tapered chunks
            o_sb = cpool.tile([H2, N_TILES * BT], _f32, tag="o")
            warm_sb = cpool.tile([128, 64], _f16, tag="warm")

            # PE pre-warm: HAM holds the PE at 1.2 GHz until ~3.4us of
            # sustained activity; it re-throttles after a >3.4us idle
            # gap. The dummy stream covers engine-ready until w1m+x0
            # land; intermittent early matmuls then keep the window hot.
            nc.vector.memset(warm_sb[:], 0)
            warm_ps = psh.tile([H1, BT], _f32, tag="ph", name="warm")
            for _ in range(WARM_MM):
                nc.tensor.matmul(
                    warm_ps[:64, :64], warm_sb[:, :64], warm_sb[:, :64],
                    start=True, stop=True,
                )

            def chunk_mm(ph, t, c, start):
                nc.tensor.matmul(
                    ph[:],
                    w1m_sb[:, c * H1 : (c + 1) * H1],
                    x_sbs[t][:, c * BT : (c + 1) * BT],
                    start=start,
                    stop=False,
                )

            def rem_mm(ph, t):
                g, q = t // 4, t % 4
                nc.tensor.matmul(
                    ph[:],
                    w1r_sb[32 * g : 32 * g + F_REM, :],
                    xr_sb[32 * g : 32 * g + F_REM, q * BT : (q + 1) * BT],
                    start=False,
                    stop=True,
                    tile_position=(96, 0) if g == 3 else None,
                )

            def relu(h, ph, scalar_eng=False):
                # Mid-kernel relus live on DVE: the scalar engine heads a
                # DMA ring, and a relu's PSUM wait there stalls every x
                # issue queued behind it. The final pair's relus DO use
                # the activation engine -- its queue is empty by then, and
                # it takes them off the vector tail chain (bias adds).
                if scalar_eng:
                    nc.scalar.activation(
                        h, ph, mybir.ActivationFunctionType.Relu, bias=b1_sb[:, 0:1]
                    )
                else:
                    nc.vector.tensor_scalar(
                        h, ph, b1_sb[:, 0:1], 0.0,
                        mybir.AluOpType.add, mybir.AluOpType.max,
                    )

            def issue_x(t, eng):
                # column halves (3072B rows, 128-row descriptors): 64-row
                # descriptors starve the engines (~300 GB/s aggregate);
                # 128-row ones sustain ~400+.
                x_sb = xpool.tile([128, FC * BT], _f16, tag="x", name="x")
                x_sbs[t] = x_sb
                src = xm[t - 2].rearrange("p c b -> p (c b)")
                eng.dma_start(out=x_sb[:, :hw], in_=src[:, :hw])
                eng.dma_start(out=x_sb[:, hw:], in_=src[:, hw:])

            # Software-pipelined schedule over pairs p = (2p, 2p+1):
            #   PE stream: A0 A1 [R0] A2 [R1 F0] ... A6 [R5 F4]
            #              [R6 F5 F6] A7 [R7] [F7]
            # A_p = 12 chunk matmuls (DMA-gated, t-major since tiles now
            # arrive one at a time), R_p = 2 remainder matmuls, F_p = 2
            # fc2 matmuls. Lag (R=1, F=2) keeps one full A-cycle between
            # relu_p and F_p while leaving only R7+F7 after the last
            # DMA-gated A-pass; pair 6's R/F is hoisted before A7 so it
            # fills the final DMA wait instead of trailing it.
            state = {}

            def emit_A(p):
                phs = [psh.tile([H1, BT], _f32, tag="ph", name="ph") for _ in range(2)]
                state[p] = {"phs": phs}
                with nc.named_scope(f"A{p}"):
                    # c-major: consecutive passes alternate the two PSUM
                    # accumulators; same-bank back-to-back accumulation
                    # costs ~63ns/pass in write-port hazards (measured)
                    for c in range(FC):
                        for k, tt in enumerate((2 * p, 2 * p + 1)):
                            chunk_mm(phs[k], tt, c, start=(c == 0))

            def emit_R(p, split_last=False):
                st = state[p]
                hs = [hpool.tile([H1, BT], _f16, tag="h", name="h") for _ in range(2)]
                st["hs"] = hs
                with nc.named_scope(f"R{p}"):
                    for k, tt in enumerate((2 * p, 2 * p + 1)):
                        rem_mm(st["phs"][k], tt)
                    relu(hs[0][:], st["phs"][0][:], scalar_eng=split_last)
                    if split_last:
                        # halves: F15h0 can start one half-relu earlier
                        hb = BT // 2
                        for j in range(2):
                            cols = slice(j * hb, (j + 1) * hb)
                            relu(
                                hs[1][:, cols], st["phs"][1][:, cols], scalar_eng=True
                            )
                    else:
                        relu(hs[1][:], st["phs"][1][:])

            def emit_F(p, split=False):
                st = state[p]
                hb = BT // 2
                with nc.named_scope(f"F{p}"):
                    if split:
                        # final pair: each matmul gets its own PSUM tile (a
                        # shared tile would serialize F15h1 behind the h0
                        # bias read), t14 leaves on sync right after its
                        # bias, t15 as one DMA on scalar after its halves
                        t = 2 * p + 1
                        po14 = pso.tile([H2, BT], _f32, tag="po", name="po")
                        nc.tensor.matmul(
                            po14[:], w2_sb[:], st["hs"][0][:], start=True, stop=True,
                        )
                        nc.vector.tensor_scalar_add(
                            o_sb[:, 14 * BT : 15 * BT], po14[:], b2_sb[:, 0:1]
                        )
                        nc.gpsimd.dma_start(
                            out=y[:, 14 * BT : 15 * BT],
                            in_=o_sb[:, 14 * BT : 15 * BT],
                        )
                        for j in range(2):
                            cols = slice(j * hb, (j + 1) * hb)
                            poh = pso.tile([H2, hb], _f32, tag="po", name="po")
                            nc.tensor.matmul(
                                poh[:], w2_sb[:], st["hs"][1][:, cols],
                                start=True, stop=True,
                            )
                            nc.vector.tensor_scalar_add(
                                o_sb[:, t * BT + j * hb : t * BT + (j + 1) * hb],
                                poh[:], b2_sb[:, 0:1],
                            )
                        nc.gpsimd.dma_start(
                            out=y[:, 15 * BT :], in_=o_sb[:, 15 * BT :]
                        )
                    else:
                        pos = [
                            pso.tile([H2, BT], _f32, tag="po", name="po")
                            for _ in range(2)
                        ]
                        for k, tt in enumerate((2 * p, 2 * p + 1)):
                            nc.tensor.matmul(
                                pos[k][:], w2_sb[:], st["hs"][k][:],
                                start=True, stop=True,
                            )
                            nc.vector.tensor_scalar_add(
                                o_sb[:, tt * BT : (tt + 1) * BT],
                                pos[k][:], b2_sb[:, 0:1],
                            )

            # two DMA queues only: each sustains ~200 GB/s (DGE
            # generation bound); a third queue makes the engines contend
            # and drops the aggregate to ~300 (measured)
            NP = N_TILES // 2  # 8 pairs
            for p in range(NP):
                if p == NP - 1:
                    # last pair: halves split ACROSS rings so both rings
                    # drain x14/x15 concurrently at the end
                    for t in (14, 15):
                        xt = xpool.tile([128, FC * BT], _f16, tag="x", name="x")
                        x_sbs[t] = xt
                        st_ = xm[t - 2].rearrange("p c b -> p (c b)")
                        nc.sync.dma_start(out=xt[:, :hw], in_=st_[:, :hw])
                        nc.scalar.dma_start(out=xt[:, hw:], in_=st_[:, hw:])
                elif p >= 1:
                    issue_x(2 * p, nc.sync)
                    issue_x(2 * p + 1, nc.scalar)
                if p == NP - 1:
                    # tapered writeback on gpsimd (idle queue): the biases
                    # these wait on are long done, nothing queues behind
                    nc.gpsimd.dma_start(out=y[:, : 4 * BT], in_=o_sb[:, : 4 * BT])
                    nc.gpsimd.dma_start(
                        out=y[:, 4 * BT : 8 * BT], in_=o_sb[:, 4 * BT : 8 * BT]
                    )

                if p == NP - 1:
                    # hoist pair-6 R/F ahead of the DMA-gated A7 passes
                    emit_R(p - 1)
                    emit_F(p - 2)
                    nc.gpsimd.dma_start(
                        out=y[:, 8 * BT : 12 * BT], in_=o_sb[:, 8 * BT : 12 * BT]
                    )
                    emit_F(p - 1)
                    nc.gpsimd.dma_start(
                        out=y[:, 12 * BT : 14 * BT], in_=o_sb[:, 12 * BT : 14 * BT]
                    )
                    emit_A(p)
                else:
                    emit_A(p)
                    if p >= 1:
                        emit_R(p - 1)
                    if p >= 2:
                        emit_F(p - 2)
            emit_R(NP - 1, split_last=True)
            emit_F(NP - 1, split=True)

    _split_sync_waits(nc)
    return nc


def _fold_conv_fc1(conv_w: np.ndarray, fc1_w: np.ndarray) -> np.ndarray:
    """Fold the 3x3 valid conv into fc1: W1eff[784, 100] such that
    h = x @ W1eff  ==  fc1( flatten(conv(x)) ).  Accumulated in float64."""
    F = fc1_w.astype(np.float64).T.reshape(26, 26, H1)
    W = np.zeros((28, 28, H1), np.float64)
    cw = conv_w.astype(np.float64)
    for di in range(3):
        for dj in range(3):
            W[di : di + 26, dj : dj + 26, :] += cw[di, dj] * F
    return W.reshape(784, H1).astype(np.float32)


def _make_in_maps(x, conv_w, fc1_w, fc1_b, fc2_w, fc2_b):
    w1eff = _fold_conv_fc1(conv_w, fc1_w)
    w1m = np.ascontiguousarray(
        w1eff[: FC * 128]
        .astype(np.float16)
        .reshape(FC, 128, H1)
        .transpose(1, 0, 2)
        .reshape(128, FC * H1)
    )
    # packed consts: cpk fp16 [128, 112] = [w1r | w2 | b1 | b2]
    cpk = np.zeros((128, CPK_W), np.float16)
    for g in range(4):
        cpk[32 * g : 32 * g + F_REM, :H1] = w1eff[FC * 128 :].astype(np.float16)
    cpk[:H1, H1 : H1 + H2] = fc2_w.T.astype(np.float16)
    cb = np.zeros((128, 2), np.float32)
    cb[:H1, 0] = fc1_b
    cb[:H2, 1] = fc2_b

    in_maps = []
    for s in range(N_CORES):
        xs = x[s * B_SHARD : (s + 1) * B_SHARD].reshape(N_TILES, BT, 784)
        xt = np.ascontiguousarray(
            xs[:, :, : FC * 128]
            .astype(np.float16)
            .reshape(N_TILES, BT, FC, 128)
            .transpose(0, 3, 2, 1)
            .reshape(N_TILES, 128, FC * BT)
        )
        xr_flat = xs.reshape(B_SHARD, 784)[:, FC * 128 :].astype(np.float16)
        xr = np.zeros((128, 4 * BT), np.float16)
        for t in range(N_TILES):
            g, q = t // 4, t % 4
            xr[32 * g : 32 * g + F_REM, q * BT : (q + 1) * BT] = xr_flat[
                t * BT : (t + 1) * BT
            ].T
        # packed head streams: mA = [w1m | x0 | xr], mB = [x1 | cpk]
        mA = np.concatenate([w1m, xt[0], xr], axis=1)
        mB = np.concatenate([xt[1], cpk], axis=1)
        xm = np.ascontiguousarray(xt[2:].reshape(N_TILES - 2, 128, FC, BT))
        in_maps.append({"xm": xm, "mA": mA, "mB": mB, "cb": cb})
    return in_maps


def _gather(results) -> np.ndarray:
    out = np.empty((B_TOTAL, H2), np.float32)
    for s in range(N_CORES):
        ys = results[s]["y"]  # [H2, B_SHARD]
        out[s * B_SHARD : (s + 1) * B_SHARD] = ys.T
    return out


def kernel_run(inputs: dict, trace: bool = False):
    """Run the kernel; returns (full output (65536,10) f32, BassKernelResults)."""
    x = np.ascontiguousarray(np.asarray(inputs["x"], dtype=np.float32))
    assert x.shape == (B_TOTAL, 784), x.shape
    in_maps = _make_in_maps(
        x,
        np.asarray(inputs["conv_w"], np.float32),
        np.asarray(inputs["fc1_w"], np.float32),
        np.asarray(inputs["fc1_b"], np.float32),
        np.asarray(inputs["fc2_w"], np.float32),
        np.asarray(inputs["fc2_b"], np.float32),
    )
    nc = _build_nc()
    res = run_bass_kernel_spmd(nc, in_maps, core_ids=list(range(N_CORES)), trace=trace)
    return _gather(res.results), res


def kernel(**inputs) -> np.ndarray:
    out, _ = kernel_run(inputs)
    return out


# revision 29
# speedup vs baseline: 1.0155x; 1.0155x over previous
"""Trainium2 Bass kernel for DigitConvolutionalModel (8-core data parallel).

Computation: x(B,784) -> 3x3 valid conv on 28x28 -> flatten(676)
             -> FC(100)+ReLU -> FC(10), B = 65536.

Algebraic restructure (host side, exact): the conv is linear, so conv and
fc1 fold into one 784->100 matrix W1eff (accumulated in float64). The
device kernel is then just two matmul layers per 512-sample tile:
  h = relu(x @ W1eff + b1);  y = h @ fc2_w.T + b2.

Numerics: the matmul datapath runs in fp16 (inputs rounded once on the
host). Measured end-to-end scale-relative absmax error vs the fp32
reference is ~4.5e-4; fp16 streams the PE at 1 col/cycle (fp32 runs at
~1/4 rate) and halves the HBM traffic, which is the kernel's bottleneck.

DMA model (measured on this part):
  - a queue keeps ~2 descriptors in flight (issue n+2 waits on n);
  - per-descriptor byte rate is descriptor-row-size bound (the DGE emits
    one packet per row; bigger rows amortize the ~105ns/packet engine
    overhead), so full-width rows (6144B) stream ~2x the rate of
    column-half rows (3072B);
  - tiny-row descriptors (cb's 8B rows, w1r's 220B rows) are pathological
    and live on the otherwise-idle GPSIMD queue, as do the y flushes
    (their bias-wait must not head-of-line block an x ring).
Every x tile therefore moves as TWO partition-row halves (rows 0:64 on
the sync ring, 64:128 on the scalar ring): full 6144B rows, both rings
loaded identically, and the last tile drains both rings concurrently.

Per-core layout (B_shard=8192 = 16 tiles x 512): x is pre-transposed on
the host to feature-major tiles so the matmul contraction lands on SBUF
partitions with no on-device transposes. Features 0..767 form 6 chunks
of 128 partitions; the 16 remainder features for all 16 tiles are packed
into one [128, 2048] tile at 32-aligned partition groups (PE row-group
granularity) and applied with per-group w1r replicas. fc1/fc2 biases ride
as two extra fp16 columns of the cpk constant block. Outputs accumulate
in SBUF and leave in tapered gpsimd writebacks so only two small writes
remain after the final tile.
"""

import numpy as np

import concourse.bass as bass
import concourse.mybir as mybir
import concourse.tile as tile
from concourse.bass_utils import run_bass_kernel_spmd
from concourse.vector_clock import ScopedClock

N_CORES = 8
B_TOTAL = 65536
B_SHARD = B_TOTAL // N_CORES  # 8192
BT = 512  # batch tile (one PSUM bank of fp32)
N_TILES = B_SHARD // BT  # 16
FC = 6  # full 128-partition feature chunks (6*128 = 768)
F_REM = 784 - FC * 128  # 16 remainder features
H1 = 100
H2 = 10

_f32 = mybir.dt.float32
_f16 = mybir.dt.float16


class SplitDrainTileContext(tile.TileContext):
    """TileContext whose tail drain carries at most one sync wait.

    The pinned walrus rejects instructions with >2 sync waits
    ("Too many sync wait commands" in setupSyncWait); the stock tail
    drain accumulates one wait per active proc. Emit one drain per
    wait instead — consecutive drains on the sync engine are
    semantically equivalent to one drain carrying all the waits.
    """

    def _drain_and_barrier(self, tick_clock, wait_clock):
        nc = self.nc
        # Cheap tail: the stock version runs two full EVSEM butterflies
        # (~13us measured). Instead: gpsimd waits on the whole vector
        # clock (all tracked incs have landed), every engine drains its
        # own DGE queues, gpsimd clears the sem ranges, and one
        # sequencer-level sem-only barrier closes the kernel.
        drain_inst = nc.gpsimd.drain()
        wait_clock.add_sem_waits(
            drain_inst.ins, ScopedClock({None: tick_clock.global_clock})
        )
        raw = drain_inst.ins
        si = raw.sync_info
        if si is not None and si.on_wait and len(si.on_wait) > 1:
            waits = list(si.on_wait)
            si.on_wait = waits[:1]
            raw.sync_info = si
            for w in waits[1:]:
                extra = nc.gpsimd.drain()
                extra.ins.sync_info = mybir.SyncInfo(on_wait=[w], on_update=[])
        for eng in (nc.sync, nc.scalar, nc.vector, nc.tensor):
            eng.drain()

        # No tail barrier: gpsimd's global-clock waits above guarantee all
        # tracked sem incs (incl. DMA completions) have landed before the
        # clears, and NRT serializes re-executions on all-engine completion.
        assert self.sems is not None
        popped = nc._tile_sem_poison_stack.pop()
        assert popped is self._sem_poison
        nc.clear_and_free_semaphores(list(self.sems.allocated().values()))


def _split_sync_waits(nc: bass.Bass, limit: int = 1) -> None:
    """Walrus-compat post-pass: the pinned walrus rejects instructions
    carrying more than ~2 sync waits. Hoist excess waits onto NoOp
    instructions inserted just before the offending instruction on the
    same engine — semantically identical (waits run in stream order)."""
    n = 0
    for fn in nc.m.functions:
        for bb in fn.blocks:
            out = []
            changed = False
            for inst in bb.instructions:
                si = inst.sync_info
                if si is not None and si.on_wait and len(si.on_wait) > limit:
                    waits = list(si.on_wait)
                    for i in range(0, len(waits) - limit, limit):
                        nop = mybir.InstNoOp(
                            name=f"swsplit-{n}",
                            ins=[],
                            outs=[],
                            sync_info=mybir.SyncInfo(
                                on_wait=waits[i : i + limit], on_update=[]
                            ),
                        )
                        nop.engine = inst.engine
                        out.append(nop)
                        n += 1
                    si.on_wait = waits[len(waits) - limit :]
                    inst.sync_info = si
                    changed = True
                out.append(inst)
            if changed:
                bb.instructions = out


WARM_MM = 48  # bridges Tensor-ready (~7.4us) to w1m+x0c0-2 landed (~10.4us)

# mA stream: [w1m | x0 | xr], f16 columns -- xr rides the main sync
# stream at full rate instead of a contending third queue
XR_W = 4 * BT
MA_W = FC * H1 + FC * BT + XR_W  # 600 + 3072 + 2048
# mB stream: [x1 | cpk], f16 columns; cpk = [w1r | w2 | b1 | b2]
CPK_W = H1 + H2 + 2
MB_W = FC * BT + CPK_W


def _build_nc() -> bass.Bass:
    nc = bass.Bass(monotonic_sem_count=0)
    # tiles 2..15 feature-major (see _make_in_maps)
    xm = nc.dram_tensor("xm", [N_TILES - 2, 128, FC, BT], _f16, kind="ExternalInput")
    mA = nc.dram_tensor("mA", [128, MA_W], _f16, kind="ExternalInput")
    mB = nc.dram_tensor("mB", [128, MB_W], _f16, kind="ExternalInput")
    # packed fp32 consts: col 0 b1 (rows 0..99), col 1 b2 (rows 0..9) --
    # tensor_scalar requires fp32 scalar operands
    cb = nc.dram_tensor("cb", [128, 2], _f32, kind="ExternalInput")
    y = nc.dram_tensor("y", [H2, N_TILES * BT], _f32, kind="ExternalOutput")

    with SplitDrainTileContext(nc) as tc:
        with (
            tc.tile_pool(name="consts", bufs=1) as cpool,
            tc.tile_pool(name="xp", bufs=12) as xpool,
            tc.tile_pool(name="hp", bufs=4) as hpool,
            tc.tile_pool(name="psh", bufs=5, space="PSUM") as psh,
            tc.tile_pool(name="pso", bufs=3, space="PSUM") as pso,
        ):
            # head: w1m first (small, clears fast), then x0/x1 as row
            # halves split across both rings. cpk (tiny rows) and xr ride
            # the gpsimd queue so they never displace an x descriptor.
            mA_sb = cpool.tile([128, MA_W], _f16, tag="mA")
            mB_sb = cpool.tile([128, MB_W], _f16, tag="mB")
            W0 = FC * H1
            hw = FC * BT // 2
            # sync's first descriptor carries w1m AND x0's first chunks in
            # one region (4272B rows): w1m alone has 1200B rows and crawls.
            # Rings drain descriptors serially, so order = priority: sync
            # then streams xr (inline mA columns) behind x0h0; scalar
            # carries x0h1 then x1.
            X0E = W0 + FC * BT
            nc.sync.dma_start(out=mA_sb[:, : W0 + hw], in_=mA[:, : W0 + hw])
            nc.scalar.dma_start(out=mA_sb[:, W0 + hw : X0E], in_=mA[:, W0 + hw : X0E])
            nc.sync.dma_start(out=mA_sb[:, X0E:], in_=mA[:, X0E:])
            nc.scalar.dma_start(out=mB_sb[:, :hw], in_=mB[:, :hw])
            nc.scalar.dma_start(out=mB_sb[:, hw : FC * BT], in_=mB[:, hw : FC * BT])
            nc.scalar.dma_start(out=mB_sb[:, FC * BT :], in_=mB[:, FC * BT :])
            cb_sb = cpool.tile([128, 2], _f32, tag="cb")
            nc.scalar.dma_start(out=cb_sb[:], in_=cb[:])
            xr_sb = mA_sb[:, X0E:]

            w1m_sb = mA_sb[:, :W0]
            cpk = mB_sb[:, FC * BT :]
            w1r_sb = cpk[:, :H1]
            w2_sb = cpk[:H1, H1 : H1 + H2]
            b1_sb = cb_sb[:H1, 0:1]
            b2_sb = cb_sb[:H2, 1:2]
            x_sbs = {0: mA_sb[:, W0:X0E], 1: mB_sb[:, : FC * BT]}

            # outputs accumulate here; written back in tapered chunks
            o_sb = cpool.tile([H2, N_TILES * BT], _f32, tag="o")
            warm_sb = cpool.tile([128, 64], _f16, tag="warm")

            # PE pre-warm: HAM holds the PE at 1.2 GHz until ~3.4us of
            # sustained activity; it re-throttles after a >3.4us idle
            # gap. The dummy stream covers engine-ready until w1m+x0
            # land; intermittent early matmuls then keep the window hot.
            nc.vector.memset(warm_sb[:], 0)
            warm_ps = psh.tile([H1, BT], _f32, tag="ph", name="warm")
            for _ in range(WARM_MM):
                nc.tensor.matmul(
                    warm_ps[:64, :64], warm_sb[:, :64], warm_sb[:, :64],
                    start=True, stop=True,
                )

            def chunk_mm(ph, t, c, start):
                nc.tensor.matmul(
                    ph[:],
                    w1m_sb[:, c * H1 : (c + 1) * H1],
                    x_sbs[t][:, c * BT : (c + 1) * BT],
                    start=start,
                    stop=False,
                )

            def rem_mm(ph, t):
                g, q = t // 4, t % 4
                nc.tensor.matmul(
                    ph[:],
                    w1r_sb[32 * g : 32 * g + F_REM, :],
                    xr_sb[32 * g : 32 * g + F_REM, q * BT : (q + 1) * BT],
                    start=False,
                    stop=True,
                    tile_position=(96, 0) if g == 3 else None,
                )

            def relu(h, ph, scalar_eng=False):
                # Mid-kernel relus live on DVE: the scalar engine heads a
                # DMA ring, and a relu's PSUM wait there stalls every x
                # issue queued behind it. The final pair's relus DO use
                # the activation engine -- its queue is empty by then, and
                # it takes them off the vector tail chain (bias adds).
                if scalar_eng:
                    nc.scalar.activation(
                        h, ph, mybir.ActivationFunctionType.Relu, bias=b1_sb[:, 0:1]
                    )
                else:
                    nc.vector.tensor_scalar(
                        h, ph, b1_sb[:, 0:1], 0.0,
                        mybir.AluOpType.add, mybir.AluOpType.max,
                    )

            def issue_x(t, eng):
                # column halves (3072B rows, 128-row descriptors): 64-row
                # descriptors starve the engines (~300 GB/s aggregate);
                # 128-row ones sustain ~400+.
                x_sb = xpool.tile([128, FC * BT], _f16, tag="x", name="x")
                x_sbs[t] = x_sb
                src = xm[t - 2].rearrange("p c b -> p (c b)")
                eng.dma_start(out=x_sb[:, :hw], in_=src[:, :hw])
                eng.dma_start(out=x_sb[:, hw:], in_=src[:, hw:])

            # Software-pipelined schedule over pairs p = (2p, 2p+1):
            #   PE stream: A0 A1 [R0] A2 [R1 F0] ... A6 [R5 F4]
            #              [R6 F5 F6] A7 [R7] [F7]
            # A_p = 12 chunk matmuls (DMA-gated, t-major since tiles now
            # arrive one at a time), R_p = 2 remainder matmuls, F_p = 2
            # fc2 matmuls. Lag (R=1, F=2) keeps one full A-cycle between
            # relu_p and F_p while leaving only R7+F7 after the last
            # DMA-gated A-pass; pair 6's R/F is hoisted before A7 so it
            # fills the final DMA wait instead of trailing it.
            state = {}

            def emit_A(p):
                phs = [psh.tile([H1, BT], _f32, tag="ph", name="ph") for _ in range(2)]
                state[p] = {"phs": phs}
                with nc.named_scope(f"A{p}"):
                    # c-major: consecutive passes alternate the two PSUM
                    # accumulators; same-bank back-to-back accumulation
                    # costs ~63ns/pass in write-port hazards (measured)
                    for c in range(FC):
                        for k, tt in enumerate((2 * p, 2 * p + 1)):
                            chunk_mm(phs[k], tt, c, start=(c == 0))

            def emit_R(p, split_last=False):
                st = state[p]
                hs = [hpool.tile([H1, BT], _f16, tag="h", name="h") for _ in range(2)]
                st["hs"] = hs
                with nc.named_scope(f"R{p}"):
                    for k, tt in enumerate((2 * p, 2 * p + 1)):
                        rem_mm(st["phs"][k], tt)
                    relu(hs[0][:], st["phs"][0][:], scalar_eng=split_last)
                    if split_last:
                        # halves: F15h0 can start one half-relu earlier
                        hb = BT // 2
                        for j in range(2):
                            cols = slice(j * hb, (j + 1) * hb)
                            relu(
                                hs[1][:, cols], st["phs"][1][:, cols], scalar_eng=True
                            )
                    else:
                        relu(hs[1][:], st["phs"][1][:])

            def emit_F(p, split=False):
                st = state[p]
                hb = BT // 2
                with nc.named_scope(f"F{p}"):
                    if split:
                        # final pair: each matmul gets its own PSUM tile (a
                        # shared tile would serialize F15h1 behind the h0
                        # bias read), t14 leaves on sync right after its
                        # bias, t15 as one DMA on scalar after its halves
                        t = 2 * p + 1
                        po14 = pso.tile([H2, BT], _f32, tag="po", name="po")
                        nc.tensor.matmul(
                            po14[:], w2_sb[:], st["hs"][0][:], start=True, stop=True,
                        )
                        nc.vector.tensor_scalar_add(
                            o_sb[:, 14 * BT : 15 * BT], po14[:], b2_sb[:, 0:1]
                        )
                        nc.gpsimd.dma_start(
                            out=y[:, 14 * BT : 15 * BT],
                            in_=o_sb[:, 14 * BT : 15 * BT],
                        )
                        for j in range(2):
                            cols = slice(j * hb, (j + 1) * hb)
                            poh = pso.tile([H2, hb], _f32, tag="po", name="po")
                            nc.tensor.matmul(
                                poh[:], w2_sb[:], st["hs"][1][:, cols],
                                start=True, stop=True,
                            )
                            nc.vector.tensor_scalar_add(
                                o_sb[:, t * BT + j * hb : t * BT + (j + 1) * hb],
                                poh[:], b2_sb[:, 0:1],
                            )
                        nc.gpsimd.dma_start(
                            out=y[:, 15 * BT :], in_=o_sb[:, 15 * BT :]
                        )
                    else:
                        pos = [
                            pso.tile([H2, BT], _f32, tag="po", name="po")
                            for _ in range(2)
                        ]
                        for k, tt in enumerate((2 * p, 2 * p + 1)):
                            nc.tensor.matmul(
                                pos[k][:], w2_sb[:], st["hs"][k][:],
                                start=True, stop=True,
                            )
                            nc.vector.tensor_scalar_add(
                                o_sb[:, tt * BT : (tt + 1) * BT],
                                pos[k][:], b2_sb[:, 0:1],
                            )

            # two DMA queues only: each sustains ~200 GB/s (DGE
            # generation bound); a third queue makes the engines contend
            # and drops the aggregate to ~300 (measured)
            NP = N_TILES // 2  # 8 pairs
            for p in range(NP):
                if p == NP - 1:
                    # last pair: halves split ACROSS rings so both rings
                    # drain x14/x15 concurrently at the end
                    for t in (14, 15):
                        xt = xpool.tile([128, FC * BT], _f16, tag="x", name="x")
                        x_sbs[t] = xt
                        st_ = xm[t - 2].rearrange("p c b -> p (c b)")
                        nc.sync.dma_start(out=xt[:, :hw], in_=st_[:, :hw])
                        nc.scalar.dma_start(out=xt[:, hw:], in_=st_[:, hw:])
                elif p >= 1:
                    issue_x(2 * p, nc.sync)
                    issue_x(2 * p + 1, nc.scalar)
                if p == NP - 1:
                    # tapered writeback on gpsimd (idle queue): the biases
                    # these wait on are long done, nothing queues behind
                    nc.gpsimd.dma_start(out=y[:, : 4 * BT], in_=o_sb[:, : 4 * BT])
                    nc.gpsimd.dma_start(
                        out=y[:, 4 * BT : 8 * BT], in_=o_sb[:, 4 * BT : 8 * BT]
                    )

                if p == NP - 1:
                    # hoist pair-6 R/F ahead of the DMA-gated A7 passes
                    emit_R(p - 1)
                    emit_F(p - 2)
                    nc.gpsimd.dma_start(
                        out=y[:, 8 * BT : 12 * BT], in_=o_sb[:, 8 * BT : 12 * BT]
                    )
                    emit_F(p - 1)
                    nc.gpsimd.dma_start(
                        out=y[:, 12 * BT : 14 * BT], in_=o_sb[:, 12 * BT : 14 * BT]
                    )
                    emit_A(p)
                else:
                    emit_A(p)
                    if p >= 1:
                        emit_R(p - 1)
                    if p >= 2:
                        emit_F(p - 2)
            emit_R(NP - 1, split_last=True)
            emit_F(NP - 1, split=True)

    _split_sync_waits(nc)
    return nc


def _fold_conv_fc1(conv_w: np.ndarray, fc1_w: np.ndarray) -> np.ndarray:
    """Fold the 3x3 valid conv into fc1: W1eff[784, 100] such that
    h = x @ W1eff  ==  fc1( flatten(conv(x)) ).  Accumulated in float64."""
    F = fc1_w.astype(np.float64).T.reshape(26, 26, H1)
    W = np.zeros((28, 28, H1), np.float64)
    cw = conv_w.astype(np.float64)
    for di in range(3):
        for dj in range(3):
            W[di : di + 26, dj : dj + 26, :] += cw[di, dj] * F
    return W.reshape(784, H1).astype(np.float32)


def _make_in_maps(x, conv_w, fc1_w, fc1_b, fc2_w, fc2_b):
    w1eff = _fold_conv_fc1(conv_w, fc1_w)
    w1m = np.ascontiguousarray(
        w1eff[: FC * 128]
        .astype(np.float16)
        .reshape(FC, 128, H1)
        .transpose(1, 0, 2)
        .reshape(128, FC * H1)
    )
    # packed consts: cpk fp16 [128, 112] = [w1r | w2 | b1 | b2]
    cpk = np.zeros((128, CPK_W), np.float16)
    for g in range(4):
        cpk[32 * g : 32 * g + F_REM, :H1] = w1eff[FC * 128 :].astype(np.float16)
    cpk[:H1, H1 : H1 + H2] = fc2_w.T.astype(np.float16)
    cb = np.zeros((128, 2), np.float32)
    cb[:H1, 0] = fc1_b
    cb[:H2, 1] = fc2_b

    in_maps = []
    for s in range(N_CORES):
        xs = x[s * B_SHARD : (s + 1) * B_SHARD].reshape(N_TILES, BT, 784)
        xt = np.ascontiguousarray(
            xs[:, :, : FC * 128]
            .astype(np.float16)
            .reshape(N_TILES, BT, FC, 128)
            .transpose(0, 3, 2, 1)
            .reshape(N_TILES, 128, FC * BT)
        )
        xr_flat = xs.reshape(B_SHARD, 784)[:, FC * 128 :].astype(np.float16)
        xr = np.zeros((128, 4 * BT), np.float16)
        for t in range(N_TILES):
            g, q = t // 4, t % 4
            xr[32 * g : 32 * g + F_REM, q * BT : (q + 1) * BT] = xr_flat[
                t * BT : (t + 1) * BT
            ].T
        # packed head streams: mA = [w1m | x0 | xr], mB = [x1 | cpk]
        mA = np.concatenate([w1m, xt[0], xr], axis=1)
        mB = np.concatenate([xt[1], cpk], axis=1)
        xm = np.ascontiguousarray(xt[2:].reshape(N_TILES - 2, 128, FC, BT))
        in_maps.append({"xm": xm, "mA": mA, "mB": mB, "cb": cb})
    return in_maps


def _gather(results) -> np.ndarray:
    out = np.empty((B_TOTAL, H2), np.float32)
    for s in range(N_CORES):
        ys = results[s]["y"]  # [H2, B_SHARD]
        out[s * B_SHARD : (s + 1) * B_SHARD] = ys.T
    return out


def kernel_run(inputs: dict, trace: bool = False):
    """Run the kernel; returns (full output (65536,10) f32, BassKernelResults)."""
    x = np.ascontiguousarray(np.asarray(inputs["x"], dtype=np.float32))
    assert x.shape == (B_TOTAL, 784), x.shape
    in_maps = _make_in_maps(
        x,
        np.asarray(inputs["conv_w"], np.float32),
        np.asarray(inputs["fc1_w"], np.float32),
        np.asarray(inputs["fc1_b"], np.float32),
        np.asarray(inputs["fc2_w"], np.float32),
        np.asarray(inputs["fc2_b"], np.float32),
    )
    nc = _build_nc()
    res = run_bass_kernel_spmd(nc, in_maps, core_ids=list(range(N_CORES)), trace=trace)
    return _gather(res.results), res


def kernel(**inputs) -> np.ndarray:
    out, _ = kernel_run(inputs)
    return out
